# revision 4
# baseline (speedup 1.0000x reference)
"""Fused pairwise-MLP kernel for Trainium2 (8 NeuronCores, SPMD data-parallel).

Computes log_q[i, j] = W3 @ gelu(W2 @ gelu(a[j] + b[i] + b1) + b2) + b3
with a = z1 @ W1a.T, b = z2 @ W1b.T  (W1 = [W1a | W1b]), N=1024, H=EMB=128.

Sharding: rows of i (z2) split across 8 cores, z1 + weights replicated
(host-side sharding; no collectives). The [N, N, H] intermediates are never
materialized in HBM — everything stays in SBUF/PSUM per 128-row i-tile.
Per core: for each of its 128 i values, one ACT gelu over [128, 1024],
two W2 matmuls, a second gelu, and an M=1 W3-dot matmul pair.
"""

import numpy as np

import concourse.bacc as bacc
import concourse.bass as bass
import concourse.tile as tile
import concourse.mybir as mybir
from concourse import bass_utils
from concourse.masks import make_identity

N = 1024
EMB = 128
HID = 128
NCORES = 8
SH = N // NCORES  # i-rows per core
F32 = mybir.dt.float32
GELU = mybir.ActivationFunctionType.Gelu


def _build():
    nc = bacc.Bacc("TRN2", target_bir_lowering=False, debug=False)

    z1_d = nc.dram_tensor("z1", (N, EMB), F32, kind="ExternalInput")
    z2_d = nc.dram_tensor("z2s", (SH, EMB), F32, kind="ExternalInput")
    w1_d = nc.dram_tensor("W1", (HID, 2 * EMB), F32, kind="ExternalInput")
    b1_d = nc.dram_tensor("b1", (HID,), F32, kind="ExternalInput")
    w2_d = nc.dram_tensor("W2", (HID, HID), F32, kind="ExternalInput")
    b2_d = nc.dram_tensor("b2", (HID,), F32, kind="ExternalInput")
    w3_d = nc.dram_tensor("W3", (1, HID), F32, kind="ExternalInput")
    b3_d = nc.dram_tensor("b3", (1,), F32, kind="ExternalInput")
    out_d = nc.dram_tensor("out", (SH, N), F32, kind="ExternalOutput")

    with tile.TileContext(nc) as tc:
        _body(tc, out_d, z1_d, z2_d, w1_d, b1_d, w2_d, b2_d, w3_d, b3_d)

    nc.compile()
    return nc


def _body(tc, out_d, z1_d, z2_d, w1_d, b1_d, w2_d, b2_d, w3_d, b3_d):
    nc = tc.nc
    with (
        tc.tile_pool(name="const", bufs=1) as const,
        tc.tile_pool(name="zload", bufs=3) as zload,
        tc.tile_pool(name="hbuf", bufs=2) as hbuf,
        tc.tile_pool(name="ps", bufs=1, space="PSUM") as ps,
        tc.tile_pool(name="apz", bufs=1, space="PSUM") as apz,
        tc.tile_pool(name="rowp", bufs=2, space="PSUM") as rowp,
    ):
        ident = const.tile([128, 128], F32)
        make_identity(nc, ident)

        # ---- load weights/biases ----
        w1_sb = const.tile([128, 2 * EMB], F32)
        nc.sync.dma_start(out=w1_sb, in_=w1_d.ap())
        w2_sb = const.tile([128, HID], F32)
        nc.sync.dma_start(out=w2_sb, in_=w2_d.ap())
        w3T_sb = const.tile([128, 1], F32)
        nc.sync.dma_start(out=w3T_sb, in_=w3_d.ap().rearrange("o h -> h o"))
        b1_sb = const.tile([128, 1], F32)
        nc.sync.dma_start(out=b1_sb, in_=b1_d.ap().rearrange("(p o) -> p o", o=1))
        b2_sb = const.tile([128, 1], F32)
        nc.sync.dma_start(out=b2_sb, in_=b2_d.ap().rearrange("(p o) -> p o", o=1))
        b3_sb = const.tile([1, 1], F32)
        nc.sync.dma_start(out=b3_sb, in_=b3_d.ap().rearrange("(p o) -> p o", o=1))

        # ---- transpose z1 -> z1T [emb, N], z2 shard -> z2T [emb, SH] ----
        z1T_sb = const.tile([128, N], F32)
        for t in range(N // 128):
            zt = zload.tile([128, 128], F32, tag="zt")
            nc.sync.dma_start(out=zt, in_=z1_d.ap()[t * 128 : (t + 1) * 128, :])
            tp = ps.tile([128, 1024], F32, tag="ps1")
            nc.tensor.transpose(tp[:, 0:128], zt, ident)
            nc.vector.tensor_copy(z1T_sb[:, t * 128 : (t + 1) * 128], tp[:, 0:128])

        z2_sb = zload.tile([128, EMB], F32, tag="zt")
        nc.sync.dma_start(out=z2_sb, in_=z2_d.ap())
        z2T_sb = const.tile([128, SH], F32)
        tp = ps.tile([128, 1024], F32, tag="ps1")
        nc.tensor.transpose(tp[:, 0:128], z2_sb, ident)
        nc.vector.tensor_copy(z2T_sb, tp[:, 0:128])

        # ---- transpose weights: W1a.T, W1b.T, W2.T (as matmul lhsT) ----
        w1aT_sb = const.tile([128, 128], F32)
        w1bT_sb = const.tile([128, 128], F32)
        w2T_sb = const.tile([128, 128], F32)
        for src, dst in (
            (w1_sb[:, 0:EMB], w1aT_sb),
            (w1_sb[:, EMB : 2 * EMB], w1bT_sb),
            (w2_sb, w2T_sb),
        ):
            tp = ps.tile([128, 1024], F32, tag="ps1")
            nc.tensor.transpose(tp[:, 0:128], src, ident)
            nc.vector.tensor_copy(dst, tp[:, 0:128])

        # ---- a[h, j] for all j; b_pp[h, i] = b[h, i] + b1 for my i-shard ----
        a_ps = apz.tile([128, N], F32)  # persistent PSUM: gelu1 streams from here
        nc.tensor.matmul(a_ps[:, 0:512], w1aT_sb, z1T_sb[:, 0:512])
        nc.tensor.matmul(a_ps[:, 512:1024], w1aT_sb, z1T_sb[:, 512:1024])

        tpb = ps.tile([128, 1024], F32, tag="ps1")
        nc.tensor.matmul(tpb[:, 0:SH], w1bT_sb, z2T_sb)
        b_pp_sb = const.tile([128, SH], F32)
        nc.vector.tensor_scalar_add(b_pp_sb, tpb[:, 0:SH], b1_sb[:, 0:1])

        # ---- main loop over my 128 i values ----
        for i in range(SH):
            h1 = hbuf.tile([128, N], F32, tag="h1")
            nc.scalar.activation(h1, a_ps, GELU, bias=b_pp_sb[:, i : i + 1])

            ps1 = ps.tile([128, N], F32, tag="ps1")
            nc.tensor.matmul(ps1[:, 0:512], w2T_sb, h1[:, 0:512])
            nc.tensor.matmul(ps1[:, 512:1024], w2T_sb, h1[:, 512:1024])

            h2 = hbuf.tile([128, N], F32, tag="h2")
            nc.scalar.activation(h2, ps1, GELU, bias=b2_sb[:, 0:1])

            pr = rowp.tile([1, N], F32, tag="row")
            nc.tensor.matmul(pr[:, 0:512], w3T_sb, h2[:, 0:512])
            nc.tensor.matmul(pr[:, 512:1024], w3T_sb, h2[:, 512:1024])
            srow = zload.tile([1, N], F32, tag="srow")
            nc.vector.tensor_scalar_add(srow, pr, b3_sb[0:1, 0:1])
            nc.sync.dma_start(out=out_d.ap()[i : i + 1, :], in_=srow)


_NC_CACHE = None


def kernel(z1, z2, W1, b1, W2, b2, W3, b3):
    global _NC_CACHE
    if _NC_CACHE is None:
        _NC_CACHE = _build()
    nc = _NC_CACHE

    z1 = np.ascontiguousarray(np.asarray(z1, dtype=np.float32))
    z2 = np.ascontiguousarray(np.asarray(z2, dtype=np.float32))
    W1 = np.ascontiguousarray(np.asarray(W1, dtype=np.float32))
    b1 = np.ascontiguousarray(np.asarray(b1, dtype=np.float32))
    W2 = np.ascontiguousarray(np.asarray(W2, dtype=np.float32))
    b2 = np.ascontiguousarray(np.asarray(b2, dtype=np.float32))
    W3 = np.ascontiguousarray(np.asarray(W3, dtype=np.float32))
    b3 = np.ascontiguousarray(np.asarray(b3, dtype=np.float32))

    in_maps = [
        {
            "z1": z1,
            "z2s": np.ascontiguousarray(z2[c * SH : (c + 1) * SH]),
            "W1": W1,
            "b1": b1,
            "W2": W2,
            "b2": b2,
            "W3": W3,
            "b3": b3,
        }
        for c in range(NCORES)
    ]
    res = bass_utils.run_bass_kernel_spmd(nc, in_maps, core_ids=list(range(NCORES)))
    return np.concatenate([r["out"] for r in res.results], axis=0)


if __name__ == "__main__":
    rng = np.random.default_rng(0)
    s1 = 1.0 / np.sqrt(2 * EMB)
    s2 = 1.0 / np.sqrt(HID)
    ins = dict(
        z1=rng.standard_normal((N, EMB), dtype=np.float32),
        z2=rng.standard_normal((N, EMB), dtype=np.float32),
        W1=rng.uniform(-s1, s1, (HID, 2 * EMB)).astype(np.float32),
        b1=rng.uniform(-s1, s1, (HID,)).astype(np.float32),
        W2=rng.uniform(-s2, s2, (HID, HID)).astype(np.float32),
        b2=rng.uniform(-s2, s2, (HID,)).astype(np.float32),
        W3=rng.uniform(-s2, s2, (1, HID)).astype(np.float32),
        b3=rng.uniform(-s2, s2, (1,)).astype(np.float32),
    )
    out = kernel(**ins)
    print("out", out.shape, out.dtype, out[:2, :4])


# revision 9
# speedup vs baseline: 1.5687x; 1.5687x over previous
"""Fused pairwise-MLP kernel for Trainium2 (8 NeuronCores, SPMD data-parallel).

Computes log_q[i, j] = W3 @ gelu(W2 @ gelu(a[j] + b[i] + b1) + b2) + b3
with a = z1 @ W1a.T, b = z2 @ W1b.T  (W1 = [W1a | W1b]), N=1024, H=EMB=128.

Sharding: rows of i (z2) split across 8 cores, z1 + weights replicated
(host-side sharding; no collectives). The [N, N, H] intermediates are never
materialized in HBM — everything stays in SBUF/PSUM per 128-row i-tile.
Per core: for each of its 128 i values, one ACT gelu over [128, 1024],
two W2 matmuls, a second gelu, and an M=1 W3-dot matmul pair.
"""

import numpy as np

import concourse.bacc as bacc
import concourse.bass as bass
import concourse.tile as tile
import concourse.mybir as mybir
from concourse import bass_utils
from concourse.masks import make_identity

N = 1024
EMB = 128
HID = 128
NCORES = 8
SH = N // NCORES  # i-rows per core
F32 = mybir.dt.float32
F32R = mybir.dt.float32r  # fp32 bits, single-pass reduced-precision matmul
GELU = mybir.ActivationFunctionType.Gelu


def _build():
    nc = bacc.Bacc("TRN2", target_bir_lowering=False, debug=False)

    z1_d = nc.dram_tensor("z1", (N, EMB), F32, kind="ExternalInput")
    z2_d = nc.dram_tensor("z2s", (SH, EMB), F32, kind="ExternalInput")
    w1_d = nc.dram_tensor("W1", (HID, 2 * EMB), F32, kind="ExternalInput")
    b1_d = nc.dram_tensor("b1", (HID,), F32, kind="ExternalInput")
    w2_d = nc.dram_tensor("W2", (HID, HID), F32, kind="ExternalInput")
    b2_d = nc.dram_tensor("b2", (HID,), F32, kind="ExternalInput")
    w3_d = nc.dram_tensor("W3", (1, HID), F32, kind="ExternalInput")
    b3_d = nc.dram_tensor("b3", (1,), F32, kind="ExternalInput")
    out_d = nc.dram_tensor("out", (SH, N), F32, kind="ExternalOutput")

    with tile.TileContext(nc) as tc:
        _body(tc, out_d, z1_d, z2_d, w1_d, b1_d, w2_d, b2_d, w3_d, b3_d)

    nc.compile()
    return nc


def _body(tc, out_d, z1_d, z2_d, w1_d, b1_d, w2_d, b2_d, w3_d, b3_d):
    nc = tc.nc
    with (
        tc.tile_pool(name="const", bufs=1) as const,
        tc.tile_pool(name="zload", bufs=3) as zload,
        tc.tile_pool(name="hbuf", bufs=2) as hbuf,
        tc.tile_pool(name="ps", bufs=1, space="PSUM") as ps,
        tc.tile_pool(name="apz", bufs=1, space="PSUM") as apz,
        tc.tile_pool(name="rowp", bufs=2, space="PSUM") as rowp,
    ):
        ident = const.tile([128, 128], F32)
        make_identity(nc, ident)

        # ---- load weights/biases ----
        w1_sb = const.tile([128, 2 * EMB], F32)
        nc.sync.dma_start(out=w1_sb, in_=w1_d.ap())
        w2_sb = const.tile([128, HID], F32)
        nc.sync.dma_start(out=w2_sb, in_=w2_d.ap())
        w3row_sb = const.tile([1, HID], F32)
        nc.sync.dma_start(out=w3row_sb, in_=w3_d.ap())
        b1_sb = const.tile([128, 1], F32)
        nc.sync.dma_start(out=b1_sb, in_=b1_d.ap().rearrange("(p o) -> p o", o=1))
        b2_sb = const.tile([128, 1], F32)
        nc.sync.dma_start(out=b2_sb, in_=b2_d.ap().rearrange("(p o) -> p o", o=1))
        b3_sb = const.tile([1, 1], F32)
        nc.sync.dma_start(out=b3_sb, in_=b3_d.ap().rearrange("(p o) -> p o", o=1))

        # ---- transpose z1 -> z1T [emb, N], z2 shard -> z2T [emb, SH] ----
        z1T_sb = const.tile([128, N], F32)
        for t in range(N // 128):
            zt = zload.tile([128, 128], F32, tag="zt")
            nc.sync.dma_start(out=zt, in_=z1_d.ap()[t * 128 : (t + 1) * 128, :])
            tp = ps.tile([128, 1024], F32, tag="ps1")
            nc.tensor.transpose(tp[:, 0:128], zt, ident)
            nc.vector.tensor_copy(z1T_sb[:, t * 128 : (t + 1) * 128], tp[:, 0:128])

        z2_sb = zload.tile([128, EMB], F32, tag="zt")
        nc.sync.dma_start(out=z2_sb, in_=z2_d.ap())
        z2T_sb = const.tile([128, SH], F32)
        tp = ps.tile([128, 1024], F32, tag="ps1")
        nc.tensor.transpose(tp[:, 0:128], z2_sb, ident)
        nc.vector.tensor_copy(z2T_sb, tp[:, 0:128])

        # ---- transpose weights: W1a.T, W1b.T, W2.T, W3.T (as matmul lhsT) ----
        w1aT_sb = const.tile([128, 128], F32)
        w1bT_sb = const.tile([128, 128], F32)
        w2T_sb = const.tile([128, 128], F32R)
        for src, dst in (
            (w1_sb[:, 0:EMB], w1aT_sb),
            (w1_sb[:, EMB : 2 * EMB], w1bT_sb),
            (w2_sb, w2T_sb),
        ):
            tp = ps.tile([128, 1024], F32, tag="ps1")
            nc.tensor.transpose(tp[:, 0:128], src, ident)
            nc.vector.tensor_copy(dst, tp[:, 0:128])

        w3T_sb = const.tile([128, 1], F32R)
        tp = ps.tile([128, 1024], F32, tag="ps1")
        nc.tensor.transpose(tp[:, 0:1], w3row_sb, ident[0:1, 0:1])
        nc.vector.tensor_copy(w3T_sb, tp[:, 0:1])

        # ---- a[h, j] for all j; b_pp[h, i] = b[h, i] + b1 for my i-shard ----
        a_ps = apz.tile([128, N], F32)  # persistent PSUM: gelu1 streams from here
        nc.tensor.matmul(a_ps[:, 0:512], w1aT_sb, z1T_sb[:, 0:512])
        nc.tensor.matmul(a_ps[:, 512:1024], w1aT_sb, z1T_sb[:, 512:1024])

        tpb = ps.tile([128, 1024], F32, tag="ps1")
        nc.tensor.matmul(tpb[:, 0:SH], w1bT_sb, z2T_sb)
        b_pp_sb = const.tile([128, SH], F32)
        nc.vector.tensor_scalar_add(b_pp_sb, tpb[:, 0:SH], b1_sb[:, 0:1])

        # ---- main loop over my 128 i values ----
        for i in range(SH):
            h1 = hbuf.tile([128, N], F32R, tag="h1")
            nc.scalar.activation(h1, a_ps, GELU, bias=b_pp_sb[:, i : i + 1])

            ps1 = ps.tile([128, N], F32, tag="ps1")
            nc.tensor.matmul(ps1[:, 0:512], w2T_sb, h1[:, 0:512])
            nc.tensor.matmul(ps1[:, 512:1024], w2T_sb, h1[:, 512:1024])

            h2 = hbuf.tile([128, N], F32R, tag="h2")
            nc.scalar.activation(h2, ps1, GELU, bias=b2_sb[:, 0:1])

            pr = rowp.tile([1, N], F32, tag="row")
            nc.tensor.matmul(pr[:, 0:512], w3T_sb, h2[:, 0:512])
            nc.tensor.matmul(pr[:, 512:1024], w3T_sb, h2[:, 512:1024])
            srow = zload.tile([1, N], F32, tag="srow")
            nc.vector.tensor_scalar_add(srow, pr, b3_sb[0:1, 0:1])
            nc.sync.dma_start(out=out_d.ap()[i : i + 1, :], in_=srow)


_NC_CACHE = None


def kernel(z1, z2, W1, b1, W2, b2, W3, b3):
    global _NC_CACHE
    if _NC_CACHE is None:
        _NC_CACHE = _build()
    nc = _NC_CACHE

    z1 = np.ascontiguousarray(np.asarray(z1, dtype=np.float32))
    z2 = np.ascontiguousarray(np.asarray(z2, dtype=np.float32))
    W1 = np.ascontiguousarray(np.asarray(W1, dtype=np.float32))
    b1 = np.ascontiguousarray(np.asarray(b1, dtype=np.float32))
    W2 = np.ascontiguousarray(np.asarray(W2, dtype=np.float32))
    b2 = np.ascontiguousarray(np.asarray(b2, dtype=np.float32))
    W3 = np.ascontiguousarray(np.asarray(W3, dtype=np.float32))
    b3 = np.ascontiguousarray(np.asarray(b3, dtype=np.float32))

    in_maps = [
        {
            "z1": z1,
            "z2s": np.ascontiguousarray(z2[c * SH : (c + 1) * SH]),
            "W1": W1,
            "b1": b1,
            "W2": W2,
            "b2": b2,
            "W3": W3,
            "b3": b3,
        }
        for c in range(NCORES)
    ]
    res = bass_utils.run_bass_kernel_spmd(nc, in_maps, core_ids=list(range(NCORES)))
    return np.concatenate([r["out"] for r in res.results], axis=0)


if __name__ == "__main__":
    rng = np.random.default_rng(0)
    s1 = 1.0 / np.sqrt(2 * EMB)
    s2 = 1.0 / np.sqrt(HID)
    ins = dict(
        z1=rng.standard_normal((N, EMB), dtype=np.float32),
        z2=rng.standard_normal((N, EMB), dtype=np.float32),
        W1=rng.uniform(-s1, s1, (HID, 2 * EMB)).astype(np.float32),
        b1=rng.uniform(-s1, s1, (HID,)).astype(np.float32),
        W2=rng.uniform(-s2, s2, (HID, HID)).astype(np.float32),
        b2=rng.uniform(-s2, s2, (HID,)).astype(np.float32),
        W3=rng.uniform(-s2, s2, (1, HID)).astype(np.float32),
        b3=rng.uniform(-s2, s2, (1,)).astype(np.float32),
    )
    out = kernel(**ins)
    print("out", out.shape, out.dtype, out[:2, :4])


# revision 18
# speedup vs baseline: 2.0181x; 1.2865x over previous
"""Fused pairwise-MLP kernel for Trainium2 (8 NeuronCores, SPMD data-parallel).

Computes log_q[i, j] = W3 @ gelu(W2 @ gelu(a[j] + b[i] + b1) + b2) + b3
with a = z1 @ W1a.T, b = z2 @ W1b.T  (W1 = [W1a | W1b]), N=1024, H=EMB=128.

Sharding: rows of i (z2) split across 8 cores, z1 + weights replicated
(host-side sharding; no collectives). The [N, N, H] intermediates are never
materialized in HBM — everything stays in SBUF/PSUM per 128-row i-tile.
Per core: for each of its 128 i values, one ACT gelu over [128, 1024],
two W2 matmuls, a second gelu, and an M=1 W3-dot matmul pair.
"""

import numpy as np

import concourse.bacc as bacc
import concourse.bass as bass
import concourse.tile as tile
import concourse.mybir as mybir
from concourse import bass_utils
from concourse.masks import make_identity

N = 1024
EMB = 128
HID = 128
NCORES = 8
SH = N // NCORES  # i-rows per core
F32 = mybir.dt.float32
F32R = mybir.dt.float32r  # fp32 bits, single-pass reduced-precision matmul
BF16 = mybir.dt.bfloat16
GELU = mybir.ActivationFunctionType.Gelu

# W3-dot implementation: "f32r" = two serial M=1 fp32r matmuls per row;
# "bf16ct" = bf16 4-way column-tiled (4 rows concurrently in the PE array).
MM2_MODE = "bf16ct"


def _build():
    nc = bacc.Bacc("TRN2", target_bir_lowering=False, debug=False)

    z1_d = nc.dram_tensor("z1", (N, EMB), F32, kind="ExternalInput")
    z2_d = nc.dram_tensor("z2s", (SH, EMB), F32, kind="ExternalInput")
    w1_d = nc.dram_tensor("W1", (HID, 2 * EMB), F32, kind="ExternalInput")
    b1_d = nc.dram_tensor("b1", (HID,), F32, kind="ExternalInput")
    w2_d = nc.dram_tensor("W2", (HID, HID), F32, kind="ExternalInput")
    b2_d = nc.dram_tensor("b2", (HID,), F32, kind="ExternalInput")
    w3_d = nc.dram_tensor("W3", (1, HID), F32, kind="ExternalInput")
    b3_d = nc.dram_tensor("b3", (1,), F32, kind="ExternalInput")
    out_d = nc.dram_tensor("out", (SH, N), F32, kind="ExternalOutput")

    with tile.TileContext(nc) as tc:
        _body(tc, out_d, z1_d, z2_d, w1_d, b1_d, w2_d, b2_d, w3_d, b3_d)

    nc.compile()
    return nc


def _body(tc, out_d, z1_d, z2_d, w1_d, b1_d, w2_d, b2_d, w3_d, b3_d):
    nc = tc.nc
    with (
        tc.tile_pool(name="const", bufs=1) as const,
        tc.tile_pool(name="zload", bufs=3) as zload,
        tc.tile_pool(name="h1p", bufs=4) as h1p,
        tc.tile_pool(name="h2p", bufs=6) as h2p,
        tc.tile_pool(name="srows", bufs=6) as srows,
        tc.tile_pool(name="ps", bufs=2, space="PSUM") as ps,
        tc.tile_pool(name="apz", bufs=1, space="PSUM") as apz,
        tc.tile_pool(name="rowp", bufs=1, space="PSUM") as rowp,
    ):
        ident = const.tile([128, 128], F32)
        make_identity(nc, ident)

        # ---- load weights/biases ----
        w1_sb = const.tile([128, 2 * EMB], F32)
        nc.sync.dma_start(out=w1_sb, in_=w1_d.ap())
        w2_sb = const.tile([128, HID], F32)
        nc.sync.dma_start(out=w2_sb, in_=w2_d.ap())
        w3row_sb = const.tile([1, HID], F32)
        nc.sync.dma_start(out=w3row_sb, in_=w3_d.ap())
        b1_sb = const.tile([128, 1], F32)
        nc.sync.dma_start(out=b1_sb, in_=b1_d.ap().rearrange("(p o) -> p o", o=1))
        b2_sb = const.tile([128, 1], F32)
        nc.sync.dma_start(out=b2_sb, in_=b2_d.ap().rearrange("(p o) -> p o", o=1))
        b3_sb = const.tile([1, 1], F32)
        nc.sync.dma_start(out=b3_sb, in_=b3_d.ap().rearrange("(p o) -> p o", o=1))

        # ---- transpose z1 -> z1T [emb, N], z2 shard -> z2T [emb, SH] ----
        z1T_sb = const.tile([128, N], F32)
        for t in range(N // 128):
            zt = zload.tile([128, 128], F32, tag="zt")
            nc.sync.dma_start(out=zt, in_=z1_d.ap()[t * 128 : (t + 1) * 128, :])
            tp = ps.tile([128, 1024], F32, tag="ps1")
            nc.tensor.transpose(tp[:, 0:128], zt, ident)
            nc.vector.tensor_copy(z1T_sb[:, t * 128 : (t + 1) * 128], tp[:, 0:128])

        z2_sb = zload.tile([128, EMB], F32, tag="zt")
        nc.sync.dma_start(out=z2_sb, in_=z2_d.ap())
        z2T_sb = const.tile([128, SH], F32)
        tp = ps.tile([128, 1024], F32, tag="ps1")
        nc.tensor.transpose(tp[:, 0:128], z2_sb, ident)
        nc.vector.tensor_copy(z2T_sb, tp[:, 0:128])

        # ---- transpose weights: W1a.T, W1b.T, W2.T, W3.T (as matmul lhsT) ----
        w1aT_sb = const.tile([128, 128], F32)
        w1bT_sb = const.tile([128, 128], F32)
        w2T_sb = const.tile([128, 128], F32R)
        for src, dst in (
            (w1_sb[:, 0:EMB], w1aT_sb),
            (w1_sb[:, EMB : 2 * EMB], w1bT_sb),
            (w2_sb, w2T_sb),
        ):
            tp = ps.tile([128, 1024], F32, tag="ps1")
            nc.tensor.transpose(tp[:, 0:128], src, ident)
            nc.vector.tensor_copy(dst, tp[:, 0:128])

        # W3.T as matmul lhsT. For bf16ct: padded to [128, 32] (cols 1-31
        # zero) so the W3-dot matmuls are valid M=32 column tiles; only row 0
        # of each 32-row block is real.
        tp = ps.tile([128, 1024], F32, tag="ps1")
        nc.tensor.transpose(tp[:, 0:1], w3row_sb, ident[0:1, 0:1])
        if MM2_MODE == "bf16ct":
            w3pad_f = const.tile([128, 32], F32)
            nc.vector.memset(w3pad_f, 0.0)
            nc.vector.tensor_copy(w3pad_f[:, 0:1], tp[:, 0:1])
            w3T_sb = const.tile([128, 32], BF16)
            nc.vector.tensor_copy(w3T_sb, w3pad_f)
        else:
            w3T_sb = const.tile([128, 1], F32R)
            nc.vector.tensor_copy(w3T_sb, tp[:, 0:1])

        # ---- a[h, j] for all j; b_pp[h, i] = b[h, i] + b1 for my i-shard ----
        a_ps = apz.tile([128, N], F32)  # persistent PSUM: gelu1 streams from here
        nc.tensor.matmul(a_ps[:, 0:512], w1aT_sb, z1T_sb[:, 0:512])
        nc.tensor.matmul(a_ps[:, 512:1024], w1aT_sb, z1T_sb[:, 512:1024])

        tpb = ps.tile([128, 1024], F32, tag="ps1")
        nc.tensor.matmul(tpb[:, 0:SH], w1bT_sb, z2T_sb)
        b_pp_sb = const.tile([128, SH], F32)
        nc.vector.tensor_scalar_add(b_pp_sb, tpb[:, 0:SH], b1_sb[:, 0:1])

        # ---- main loop over my 128 i values ----
        # Software-pipelined emission: gelu1 runs 2 iterations ahead so the
        # ACT stream (g1(i+2), g2(i), g1(i+3), g2(i+1), ...) never stalls on
        # the W2 matmuls. mm2 (M=1 W3 dot) is batched per 4 i's into the 4
        # PE column-groups (outputs at PSUM partitions 0/32/64/96) so the 4
        # matmuls execute concurrently in the array.
        h1s = [None] * SH

        def emit_g1(i):
            h1s[i] = h1p.tile([128, N], F32R, tag="h1", name="h1")
            nc.scalar.activation(h1s[i], a_ps, GELU, bias=b_pp_sb[:, i : i + 1])

        emit_g1(0)
        emit_g1(1)
        h2s = [None] * 4
        for i in range(SH):
            if i + 2 < SH:
                emit_g1(i + 2)

            ps1 = ps.tile([128, N], F32, tag="ps1")
            nc.tensor.matmul(ps1[:, 0:512], w2T_sb, h1s[i][:, 0:512])
            nc.tensor.matmul(ps1[:, 512:1024], w2T_sb, h1s[i][:, 512:1024])
            h1s[i] = None

            h2 = h2p.tile([128, N], BF16 if MM2_MODE == "bf16ct" else F32R, tag="h2")
            nc.scalar.activation(h2, ps1, GELU, bias=b2_sb[:, 0:1])
            h2s[i % 4] = h2

            if MM2_MODE == "bf16ct" and i % 4 == 3:
                prA = rowp.tile([128, 512], F32, tag="rowA")
                prB = rowp.tile([128, 512], F32, tag="rowB")
                for k in range(4):
                    nc.tensor.matmul(
                        prA[32 * k : 32 * k + 32, :],
                        w3T_sb,
                        h2s[k][:, 0:512],
                        tile_position=(0, 32 * k),
                    )
                    nc.tensor.matmul(
                        prB[32 * k : 32 * k + 32, :],
                        w3T_sb,
                        h2s[k][:, 512:1024],
                        tile_position=(0, 32 * k),
                    )
                for k in range(4):
                    ii = i - 3 + k
                    srow = srows.tile([1, N], F32, tag="srow")
                    nc.vector.tensor_scalar_add(
                        srow[:, 0:512], prA[32 * k : 32 * k + 1, :], b3_sb[0:1, 0:1]
                    )
                    nc.vector.tensor_scalar_add(
                        srow[:, 512:1024], prB[32 * k : 32 * k + 1, :], b3_sb[0:1, 0:1]
                    )
                    nc.sync.dma_start(out=out_d.ap()[ii : ii + 1, :], in_=srow)
            elif MM2_MODE == "f32r":
                pr = rowp.tile([1, N], F32, tag="rowA")
                nc.tensor.matmul(pr[:, 0:512], w3T_sb, h2[:, 0:512])
                nc.tensor.matmul(pr[:, 512:1024], w3T_sb, h2[:, 512:1024])
                srow = srows.tile([1, N], F32, tag="srow")
                nc.vector.tensor_scalar_add(srow, pr, b3_sb[0:1, 0:1])
                nc.sync.dma_start(out=out_d.ap()[i : i + 1, :], in_=srow)


_NC_CACHE = None


def kernel(z1, z2, W1, b1, W2, b2, W3, b3):
    global _NC_CACHE
    if _NC_CACHE is None:
        _NC_CACHE = _build()
    nc = _NC_CACHE

    z1 = np.ascontiguousarray(np.asarray(z1, dtype=np.float32))
    z2 = np.ascontiguousarray(np.asarray(z2, dtype=np.float32))
    W1 = np.ascontiguousarray(np.asarray(W1, dtype=np.float32))
    b1 = np.ascontiguousarray(np.asarray(b1, dtype=np.float32))
    W2 = np.ascontiguousarray(np.asarray(W2, dtype=np.float32))
    b2 = np.ascontiguousarray(np.asarray(b2, dtype=np.float32))
    W3 = np.ascontiguousarray(np.asarray(W3, dtype=np.float32))
    b3 = np.ascontiguousarray(np.asarray(b3, dtype=np.float32))

    in_maps = [
        {
            "z1": z1,
            "z2s": np.ascontiguousarray(z2[c * SH : (c + 1) * SH]),
            "W1": W1,
            "b1": b1,
            "W2": W2,
            "b2": b2,
            "W3": W3,
            "b3": b3,
        }
        for c in range(NCORES)
    ]
    res = bass_utils.run_bass_kernel_spmd(nc, in_maps, core_ids=list(range(NCORES)))
    return np.concatenate([r["out"] for r in res.results], axis=0)


if __name__ == "__main__":
    rng = np.random.default_rng(0)
    s1 = 1.0 / np.sqrt(2 * EMB)
    s2 = 1.0 / np.sqrt(HID)
    ins = dict(
        z1=rng.standard_normal((N, EMB), dtype=np.float32),
        z2=rng.standard_normal((N, EMB), dtype=np.float32),
        W1=rng.uniform(-s1, s1, (HID, 2 * EMB)).astype(np.float32),
        b1=rng.uniform(-s1, s1, (HID,)).astype(np.float32),
        W2=rng.uniform(-s2, s2, (HID, HID)).astype(np.float32),
        b2=rng.uniform(-s2, s2, (HID,)).astype(np.float32),
        W3=rng.uniform(-s2, s2, (1, HID)).astype(np.float32),
        b3=rng.uniform(-s2, s2, (1,)).astype(np.float32),
    )
    out = kernel(**ins)
    print("out", out.shape, out.dtype, out[:2, :4])
